# revision 6
# baseline (speedup 1.0000x reference)
"""Trainium2 Bass kernel for nn_DifferenceEncodingGRU.

Data-parallel over batch: B=128 sharded as 16 rows per core across 8 cores.
The small per-core state (h.T, delta.T, v.T) is the STATIONARY matmul operand
(lhsT, [K,16]); the big fixed weights are the MOVING operand streamed through
the PE in fp32r (full-rate 4-byte mode at N>=256).

Precision: x_t grows to ~1e3, so x @ Wxd.T in reduced-precision fp32r would
dominate the error. Instead GX = x_{t-1} @ Wxd.T + biases is kept as a running
fp32 SBUF accumulator, updated per step with delta = v + b_out + y (|delta|~2)
matmuls, and added into the gate pre-activations on the vector engines.

Iteration t = 0..S (one-step phase shift):
  P1 (t>0): Y = h_{t-1} @ W_out.T                  (8 accum MMs, N=64)
  P2 (t>0): delta_{t-1} = Y + vb_{t-1}; x_{t-1} = x_{t-2} + delta;
            transpose delta -> dT; GX += dT-pair MMs (r|z|n regions)
  P3 (t<S): v-pair MMs (K=64) into gate psum regions
  P4 (t<S): 8 h-chunk MMs (K=128) + elementwise (GX folded in) -> h_t
  P5 (t<S): 8 PE transposes h_t -> hT

PSUM (8 banks): RZ[16,2048](4) + HN[16,1024](2) + scratch slot(2) shared in
time by Y -> dT_ps -> gx_ps x3 -> IN -> hT_ps.
"""
import sys
sys.path.insert(0, '/opt/trn_rl_repo')
import numpy as np

H = 1024
I = 64
O = 64
B = 128
S = 512
NC = 8
BL = B // NC          # 16 batch rows per core
G = 16                # steps per DMA group

_cache = {}


def _build(s_steps=S):
    from concourse import bacc, tile, mybir
    f32 = mybir.dt.float32
    f32r = mybir.dt.float32r
    Sigmoid = mybir.ActivationFunctionType.Sigmoid
    Tanh = mybir.ActivationFunctionType.Tanh
    Copy = mybir.ActivationFunctionType.Copy
    mult = mybir.AluOpType.mult
    add = mybir.AluOpType.add
    subtract = mybir.AluOpType.subtract

    nc = bacc.Bacc("TRN2", target_bir_lowering=False, debug=False, num_devices=NC)
    d_wh = nc.dram_tensor("wh", [128, 8 * 3 * H], f32r, kind="ExternalInput")
    d_wo = nc.dram_tensor("wo", [128, 8 * O], f32r, kind="ExternalInput")
    d_wv = nc.dram_tensor("wv", [64, 3 * H], f32r, kind="ExternalInput")
    d_wx = nc.dram_tensor("wx", [64, 3 * H], f32r, kind="ExternalInput")
    d_vt = nc.dram_tensor("vt", [64, s_steps * BL], f32r, kind="ExternalInput")
    d_vb = nc.dram_tensor("vb", [BL, s_steps * O], f32r, kind="ExternalInput")
    d_gx0 = nc.dram_tensor("gx0", [BL, 4 * H], f32, kind="ExternalInput")
    d_x0 = nc.dram_tensor("x0", [BL, O], f32r, kind="ExternalInput")
    d_ident = nc.dram_tensor("ident", [16, 16], f32r, kind="ExternalInput")
    d_zz = nc.dram_tensor("zz", [128, H], f32r, kind="ExternalInput")
    d_out = nc.dram_tensor("out", [BL, s_steps * O], f32r, kind="ExternalOutput")

    with tile.TileContext(nc) as tc:
        with tc.tile_pool(name="wpool", bufs=1) as wp, \
             tc.tile_pool(name="state", bufs=1) as st, \
             tc.tile_pool(name="vtp", bufs=2) as vtp, \
             tc.tile_pool(name="vbp", bufs=2) as vbp, \
             tc.tile_pool(name="xsp", bufs=2) as xsp, \
             tc.tile_pool(name="work", bufs=1) as wk, \
             tc.tile_pool(name="ps_rz", bufs=1, space="PSUM") as prz, \
             tc.tile_pool(name="ps_hn", bufs=1, space="PSUM") as phn, \
             tc.tile_pool(name="ps_scr", bufs=1, space="PSUM") as pscr:

            wh = wp.tile([128, 8 * 3 * H], f32r, name="wh_sb")
            wo = wp.tile([128, 8 * O], f32r, name="wo_sb")
            wv = wp.tile([64, 3 * H], f32r, name="wv_sb")
            wx = wp.tile([64, 3 * H], f32r, name="wx_sb")
            ident = wp.tile([16, 16], f32r, name="ident_sb")
            nc.sync.dma_start(wh[:], d_wh[:])
            nc.sync.dma_start(wo[:], d_wo[:])
            nc.sync.dma_start(wv[:], d_wv[:])
            nc.sync.dma_start(wx[:], d_wx[:])
            nc.sync.dma_start(ident[:], d_ident[:])

            h_sb = st.tile([BL, H], f32r, name="h_sb")
            hT = st.tile([128, 128], f32r, name="hT_sb")
            GX = st.tile([BL, 4 * H], f32, name="gx_sb")
            dT = st.tile([64, BL], f32r, name="dT_sb")
            x0s = st.tile([BL, O], f32r, name="x0_sb")
            nc.sync.dma_start(h_sb[:], d_zz[0:BL, 0:H])
            nc.sync.dma_start(hT[:], d_zz[:, 0:128])
            nc.sync.dma_start(GX[:], d_gx0[:])
            nc.sync.dma_start(x0s[:], d_x0[:])

            vt_t = vb_t = xs_t = None
            vb_prev = xs_prev = None

            for t in range(s_steps + 1):
                if t < s_steps and t % G == 0:
                    g = t // G
                    n = min(G, s_steps - t)
                    vb_prev = vb_t
                    vt_t = vtp.tile([64, G * BL], f32r, name=f"vt_{g % 2}", tag="vt")
                    nc.sync.dma_start(vt_t[:, 0:n * BL],
                                      d_vt[:, g * G * BL:(g * G + n) * BL])
                    vb_t = vbp.tile([BL, G * O], f32r, name=f"vb_{g % 2}", tag="vb")
                    nc.sync.dma_start(vb_t[:, 0:n * O],
                                      d_vb[:, g * G * O:(g * G + n) * O])
                if t > 0 and (t - 1) % G == 0:
                    xs_prev = xs_t
                    xs_t = xsp.tile([BL, G * O], f32r, name=f"xs_{((t - 1) // G) % 2}", tag="xs")

                cur_vb_g = t // G if t < s_steps else (s_steps - 1) // G
                cur_xs_g = (t - 1) // G

                def xslice(tt):
                    if tt < 0:
                        return x0s[:]
                    tl = xs_t if (tt // G) == cur_xs_g else xs_prev
                    return tl[:, (tt % G) * O:(tt % G) * O + O]

                # ---- P1 + P2 ----
                if t > 0:
                    tm1 = t - 1
                    Y = pscr.tile([BL, O], f32, name=f"Y_{t}", tag="scr")
                    for k in range(8):
                        nc.tensor.matmul(Y[:], hT[:, k * 16:k * 16 + 16],
                                         wo[:, k * O:(k + 1) * O],
                                         start=(k == 0), stop=(k == 7))
                    vbsrc = vb_t if (tm1 // G) == cur_vb_g else vb_prev
                    vb_off = (tm1 % G) * O
                    dlt = wk.tile([BL, O], f32r, name=f"dlt_{t % 2}", tag="dlt")
                    nc.vector.tensor_tensor(dlt[:], Y[:], vbsrc[:, vb_off:vb_off + O], add)
                    xsl = xslice(tm1)
                    nc.vector.tensor_tensor(xsl, xslice(tm1 - 1), dlt[:], add)
                    if t < s_steps:
                        dT_ps = pscr.tile([64, BL], f32r, name=f"dTp_{t}", tag="scr")
                        nc.tensor.transpose(dT_ps[:], dlt[:], ident[:])
                        nc.scalar.activation(dT[:], dT_ps[:], Copy)
                        for rg, (wxc, gxc) in enumerate([(0, 0), (H, H), (2 * H, 3 * H)]):
                            gxp = pscr.tile([BL, H], f32, name=f"gxp_{t}_{rg}", tag="scr")
                            nc.tensor.matmul(gxp[:, 0:512], dT[:], wx[:, wxc:wxc + 512],
                                             start=True, stop=True)
                            nc.tensor.matmul(gxp[:, 512:1024], dT[:],
                                             wx[:, wxc + 512:wxc + 1024],
                                             start=True, stop=True)
                            nc.vector.tensor_tensor(GX[:, gxc:gxc + H],
                                                    GX[:, gxc:gxc + H], gxp[:], add)
                    if (t - 1) % G == G - 1 or t == s_steps:
                        g = (t - 1) // G
                        base = g * G * O
                        n = min(G * O, s_steps * O - base)
                        nc.sync.dma_start(d_out[:, base:base + n], xs_t[:, 0:n])

                if t >= s_steps:
                    continue

                # ---- P3 + P4 ----
                RZ = prz.tile([BL, 2 * H], f32, name=f"RZ_{t}", tag="rz")
                HN = phn.tile([BL, H], f32, name=f"HN_{t}", tag="hn")
                IN = pscr.tile([BL, H], f32, name=f"IN_{t}", tag="scr")
                v_lhs = vt_t[:, (t % G) * BL:(t % G) * BL + BL]

                def bank_mms(psum_ap, wv_col, wh_col, has_v):
                    if has_v:
                        nc.tensor.matmul(psum_ap, v_lhs, wv[:, wv_col:wv_col + 512],
                                         start=True, stop=False)
                    for k in range(8):
                        nc.tensor.matmul(psum_ap, hT[:, k * 16:k * 16 + 16],
                                         wh[:, k * 3 * H + wh_col:k * 3 * H + wh_col + 512],
                                         start=(not has_v) and k == 0, stop=(k == 7))

                rs = wk.tile([BL, H], f32r, name=f"rs_{t % 2}", tag="rs")
                zs = wk.tile([BL, H], f32r, name=f"zs_{t % 2}", tag="zs")
                hn2 = wk.tile([BL, H], f32r, name=f"hn2_{t % 2}", tag="hn2")
                in2 = wk.tile([BL, H], f32r, name=f"in2_{t % 2}", tag="in2")
                t1 = wk.tile([BL, H], f32r, name=f"t1_{t % 2}", tag="t1")
                t2 = wk.tile([BL, H], f32r, name=f"t2_{t % 2}", tag="t2")
                t3 = wk.tile([BL, H], f32r, name=f"t3_{t % 2}", tag="t3")
                ns = wk.tile([BL, H], f32r, name=f"ns_{t % 2}", tag="ns")
                dd = wk.tile([BL, H], f32r, name=f"dd_{t % 2}", tag="dd")
                ee = wk.tile([BL, H], f32r, name=f"ee_{t % 2}", tag="ee")

                for half in range(2):
                    c = half * 512
                    sl = slice(c, c + 512)
                    bank_mms(RZ[:, c:c + 512], c, c, True)                  # r
                    bank_mms(HN[:, c:c + 512], 0, 2 * H + c, False)         # hn
                    nc.tensor.matmul(IN[:, c:c + 512], v_lhs,
                                     wv[:, 2 * H + c:2 * H + c + 512],
                                     start=True, stop=True)                 # in
                    bank_mms(RZ[:, H + c:H + c + 512], H + c, H + c, True)  # z
                    nc.vector.tensor_tensor(t3[:, sl], RZ[:, c:c + 512], GX[:, c:c + 512], add)
                    nc.scalar.activation(rs[:, sl], t3[:, sl], Sigmoid)
                    nc.vector.tensor_tensor(hn2[:, sl], HN[:, sl],
                                            GX[:, 2 * H + c:2 * H + c + 512], add)
                    nc.vector.tensor_tensor(in2[:, sl], IN[:, sl],
                                            GX[:, 3 * H + c:3 * H + c + 512], add)
                    nc.vector.tensor_tensor(t1[:, sl], rs[:, sl], hn2[:, sl], mult)
                    nc.vector.tensor_tensor(t2[:, sl], t1[:, sl], in2[:, sl], add)
                    nc.scalar.activation(ns[:, sl], t2[:, sl], Tanh)
                    nc.vector.tensor_tensor(t3[:, sl], RZ[:, H + c:H + c + 512],
                                            GX[:, H + c:H + c + 512], add)
                    nc.scalar.activation(zs[:, sl], t3[:, sl], Sigmoid)
                    nc.vector.tensor_tensor(dd[:, sl], h_sb[:, sl], ns[:, sl], subtract)
                    nc.vector.tensor_tensor(ee[:, sl], dd[:, sl], zs[:, sl], mult)
                    nc.vector.tensor_tensor(h_sb[:, sl], ee[:, sl], ns[:, sl], add)

                # ---- P5 ----
                hT_ps = pscr.tile([128, 128], f32r, name=f"hTp_{t}", tag="scr")
                for k in range(8):
                    nc.tensor.transpose(hT_ps[:, k * 16:k * 16 + 16],
                                        h_sb[:, k * 128:(k + 1) * 128], ident[:])
                nc.scalar.activation(hT[:], hT_ps[:], Copy)

    nc.compile()
    return nc


def _prep_inputs(X0, V, W_ih, W_hh, b_ih, b_hh, W_out, b_out, s_steps=S):
    f = np.float32
    X0 = np.asarray(X0, f); V = np.asarray(V, f)
    W_ih = np.asarray(W_ih, f); W_hh = np.asarray(W_hh, f)
    b_ih = np.asarray(b_ih, f); b_hh = np.asarray(b_hh, f)
    W_out = np.asarray(W_out, f); b_out = np.asarray(b_out, f)

    W_v = W_ih[:, :I]; W_x = W_ih[:, I:I + O]; W_d = W_ih[:, I + O:]
    Wvd = W_v + W_d
    Wxd = W_x + W_d

    wh = np.ascontiguousarray(
        W_hh.T.reshape(8, 128, 3 * H).transpose(1, 0, 2).reshape(128, 8 * 3 * H))
    wo = np.ascontiguousarray(
        W_out.T.reshape(8, 128, O).transpose(1, 0, 2).reshape(128, 8 * O))
    wv = np.ascontiguousarray(Wvd.T)
    wx = np.ascontiguousarray(Wxd.T)
    ident = np.eye(16, dtype=f)

    in_maps = []
    for c in range(NC):
        sl = slice(c * BL, (c + 1) * BL)
        Vc = V[sl, :s_steps, :]
        X0c = X0[sl]
        vt = np.ascontiguousarray(Vc.transpose(2, 1, 0).reshape(64, s_steps * BL))
        vb = np.ascontiguousarray((Vc + b_out).reshape(BL, s_steps * O))
        gi0 = (X0c.astype(np.float64) @ Wxd.T.astype(np.float64)).astype(f)
        gx0 = np.zeros((BL, 4 * H), f)
        gx0[:, 0:H] = gi0[:, 0:H] + b_ih[0:H] + b_hh[0:H]
        gx0[:, H:2 * H] = gi0[:, H:2 * H] + b_ih[H:2 * H] + b_hh[H:2 * H]
        gx0[:, 2 * H:3 * H] = b_hh[2 * H:3 * H]
        gx0[:, 3 * H:4 * H] = gi0[:, 2 * H:3 * H] + b_ih[2 * H:3 * H]
        in_maps.append({
            "wh": wh, "wo": wo, "wv": wv, "wx": wx, "ident": ident,
            "zz": np.zeros((128, H), f),
            "vt": vt, "vb": vb, "gx0": gx0, "x0": np.ascontiguousarray(X0c),
        })
    return in_maps


def kernel(X0, V, W_ih, W_hh, b_ih, b_hh, W_out, b_out):
    from concourse.bass_utils import run_bass_kernel_spmd
    s_steps = V.shape[1]
    if s_steps not in _cache:
        _cache[s_steps] = _build(s_steps)
    nc = _cache[s_steps]
    in_maps = _prep_inputs(X0, V, W_ih, W_hh, b_ih, b_hh, W_out, b_out, s_steps)
    res = run_bass_kernel_spmd(nc, in_maps, core_ids=list(range(NC)))
    outs = []
    for c in range(NC):
        outs.append(res.results[c]["out"].reshape(BL, s_steps, O))
    return np.concatenate(outs, axis=0).astype(np.float32)
